# revision 25
# baseline (speedup 1.0000x reference)
"""GPT2 attention (B=2,S=2048,E=1024,H=16) on 8 NeuronCores — interleaved.

Sharding: core c -> batch b=c//4, head-group g=c%4 (4 heads, d'=256 cols).
Per-core partial c_proj outputs (bf16) are summed on the host (+b_proj).

Single interleaved instruction stream per core so the PE never starves while
the ACT engine runs softmax exp (~80us of exp vs ~113us of matmul rows):
  - inputs DMA'd in 128KB column chunks, issues spread over the 3 DMA-capable
    queues (sync/scalar/gpsimd) so the first projection can start at ~5us
  - V proj / QK^T proj / c_proj emitted as small "units" (8 or 2 matmuls +
    one DVE tail); units are force-issued when a q-chunk needs them and
    otherwise popped as PE filler between attention k-pair iterations;
    c_proj units are hoarded for the ACT-bound qc>=2 stretches
  - attention per (qc, hp): scores -> exp -> mask -> attnV with one-kp
    lookahead (attnV for kp runs while exp for kp+1 is in flight)
  - softmax denominator Z comes free via a ones-column appended to V
    (row 64 of the attnV psum); normalize = recip + gpsimd partition
    broadcast + DVE multiply, off the PE critical path
Fine-grained causal: diagonal k-tiles restrict scores/exp/attnV to the valid
column range; only the [128,128] triangle block gets a mask multiply.

Perf: ~177us HW exec (baseline 257us). PE is the critical engine at ~76%
occupancy; remaining slack is p-state ramp after small stalls, the
DMA-bound head, and the final normalize + c_proj + DMA-drain tail.
"""

import numpy as np
from collections import deque

import concourse.bass as bass
import concourse.mybir as mybir
import concourse.tile as tile
from concourse import bacc
from concourse.bass_utils import run_bass_kernel_spmd

B, S, E, H = 2, 2048, 1024, 16
HD = 64           # head dim
HPC = 4           # heads per core
DP = HPC * HD     # 256 d' columns per core
NQC = 4           # q-chunks of 512
NET = E // 128    # 8 E-tiles

f32 = mybir.dt.float32
bf16 = mybir.dt.bfloat16
f8 = mybir.dt.float8e4
f32r = mybir.dt.float32r
FT = mybir.ActivationFunctionType

_CACHED = {}


def build_nc():
    nc = bacc.Bacc("TRN2", target_bir_lowering=False, debug=False,
                   enable_asserts=False, num_devices=8)

    xT = nc.dram_tensor("xT", [E, S], bf16, kind="ExternalInput")
    wqk = nc.dram_tensor("wqk", [E, 2 * DP], bf16, kind="ExternalInput")
    bqk = nc.dram_tensor("bqk", [128, 4], f32, kind="ExternalInput")
    wv = nc.dram_tensor("wv", [E, 260], bf16, kind="ExternalInput")
    vb = nc.dram_tensor("vb", [128, 260], f32, kind="ExternalInput")
    wp = nc.dram_tensor("wp", [DP, E], bf16, kind="ExternalInput")
    mtri = nc.dram_tensor("mtri", [128, 128], bf16, kind="ExternalInput")
    outp = nc.dram_tensor("outp", [S, E], bf16, kind="ExternalOutput")

    with tile.TileContext(nc) as tc:
        with (
            nc.allow_low_precision("bf16 data, approx reciprocal"),
            tc.tile_pool(name="consts", bufs=1) as consts,
            tc.tile_pool(name="acts", bufs=1) as acts,
            tc.tile_pool(name="slabs", bufs=6) as slabs,
            tc.tile_pool(name="small", bufs=4) as small,
            tc.tile_pool(name="outs", bufs=6) as outs,
            tc.tile_pool(name="bigps", bufs=2, space="PSUM") as bigps,
            tc.tile_pool(name="otps", bufs=2, space="PSUM") as otps,
            tc.tile_pool(name="aux", bufs=2, space="PSUM") as aux,
        ):
            # ---- SBUF tiles ----
            xts = [[consts.tile([128, 512], bf16, tag=f"xt{kt}_{sc}", name=f"xt{kt}_{sc}")
                    for sc in range(4)] for kt in range(NET)]
            wqk_sb = [consts.tile([128, 2 * DP], bf16, tag=f"wqk{kt}", name=f"wqk{kt}")
                      for kt in range(NET)]
            wv_sb = [consts.tile([128, 260], bf16, tag=f"wv{kt}", name=f"wv{kt}")
                     for kt in range(NET)]
            vb_sb = consts.tile([128, 260], f32, tag="vb")
            bqk_sb = consts.tile([128, 4], f32, tag="bqk")
            mtri_sb = consts.tile([128, 128], bf16, tag="mtri")
            wp_sb = [consts.tile([128, E], bf16, tag=f"wp{t}", name=f"wp{t}") for t in range(2)]

            v_sb = [acts.tile([128, 260], bf16, tag=f"v{st}", name=f"v{st}")
                    for st in range(16)]
            qkt_sb = [acts.tile([128, S], bf16, tag=f"qkt{t}", name=f"qkt{t}")
                      for t in range(4)]
            attnT_sb = [acts.tile([128, S], bf16, tag=f"attnT{t}",
                                  name=f"attnT{t}") for t in range(2)]

            # ---- input DMAs, issues spread over 4 queues ----
            # critical first: wv + vb (V units), xts sc0, wqk + bqk (QK units)
            qs = [nc.sync, nc.scalar, nc.gpsimd]
            for kt in range(NET):
                q = qs[kt % 3]
                q.dma_start(wv_sb[kt][:], wv[kt * 128:(kt + 1) * 128, :])
                q.dma_start(xts[kt][0][:], xT[kt * 128:(kt + 1) * 128, 0:512])
            nc.gpsimd.dma_start(vb_sb[:], vb[:, :])
            for kt in range(NET):
                q = qs[kt % 3]
                q.dma_start(wqk_sb[kt][:], wqk[kt * 128:(kt + 1) * 128, :])
            nc.scalar.dma_start(bqk_sb[:], bqk[:, :])
            nc.scalar.dma_start(mtri_sb[:], mtri[:, :])
            for sc in range(1, 4):
                for kt in range(NET):
                    nc.sync.dma_start(
                        xts[kt][sc][:],
                        xT[kt * 128:(kt + 1) * 128, sc * 512:(sc + 1) * 512])
            for t in range(2):
                nc.sync.dma_start(wp_sb[t][:], wp[t * 128:(t + 1) * 128, :])

            # ---- work units ----
            done = set()
            cp_flip = [0]

            def emit_qk(t, sc):
                if ("qk", t, sc) in done:
                    return 0
                done.add(("qk", t, sc))
                qps = aux.tile([128, 512], f32, tag="aux")
                for kt in range(NET):
                    nc.tensor.matmul(
                        qps[:],
                        wqk_sb[kt][:, t * 128:(t + 1) * 128],
                        xts[kt][sc][:],
                        start=(kt == 0), stop=(kt == NET - 1))
                nc.vector.tensor_scalar_add(
                    qkt_sb[t][:, sc * 512:(sc + 1) * 512],
                    qps[:], bqk_sb[:, t:t + 1])
                return 1707

            def emit_v(st):
                if ("v", st) in done:
                    return 0
                done.add(("v", st))
                vps = aux.tile([128, 512], f32, tag="aux")
                sc, o = st // 4, (st % 4) * 128
                for kt in range(NET):
                    nc.tensor.matmul(
                        vps[:, 0:260],
                        xts[kt][sc][:, o:o + 128],
                        wv_sb[kt][:],
                        start=(kt == 0), stop=(kt == NET - 1))
                nc.vector.tensor_add(v_sb[st][:], vps[:, 0:260], vb_sb[:])
                return 867

            def emit_cp(st, nchk):
                if ("cp", st, nchk) in done:
                    return 0
                done.add(("cp", st, nchk))
                cps = aux.tile([128, 512], f32, tag="aux")
                for k2 in (0, 1):
                    nc.tensor.matmul(
                        cps[:],
                        attnT_sb[k2][:, st * 128:(st + 1) * 128],
                        wp_sb[k2][:, nchk * 512:(nchk + 1) * 512],
                        start=(k2 == 0), stop=(k2 == 1))
                ob = outs.tile([128, 512], bf16, tag="ob")
                nc.vector.tensor_copy(ob[:], cps[:])
                if st >= 12:
                    nc.sync.dma_start(
                        outp[st * 128:(st + 1) * 128,
                             nchk * 512:nchk * 512 + 256], ob[:, 0:256])
                    nc.gpsimd.dma_start(
                        outp[st * 128:(st + 1) * 128,
                             nchk * 512 + 256:(nchk + 1) * 512],
                        ob[:, 256:512])
                else:
                    (nc.gpsimd if cp_flip[0] % 2 else nc.sync).dma_start(
                        outp[st * 128:(st + 1) * 128,
                             nchk * 512:(nchk + 1) * 512], ob[:])
                cp_flip[0] += 1
                return 426

            def emit_unit(u):
                if u[0] == "qk":
                    return emit_qk(u[1], u[2])
                if u[0] == "v":
                    return emit_v(u[1])
                return emit_cp(u[1], u[2])

            work = deque()
            cpwork = deque()

            def pop_filler(budget, allow_cp):
                spent = 0
                while work and spent < budget:
                    spent += emit_unit(work.popleft())
                if allow_cp:
                    while len(cpwork) > 8 and spent < budget:
                        spent += emit_unit(cpwork.popleft())

            # ---- attention ----
            def emit_attnv(qc, hp, ots, cur):
                nkt = 4 * qc + 4
                for i, (slab, kp) in enumerate(cur):
                    h = 2 * hp + i
                    for half in (0, 1):
                        kt = 2 * kp + half
                        di = kt - 4 * qc
                        off = 128 * di if di > 0 else 0
                        nc.tensor.matmul(
                            ots[i][:, off:512],
                            v_sb[kt][:, 65 * h:65 * h + 65],
                            slab[:, half * 512 + off:(half + 1) * 512],
                            start=(kt == 0), stop=(kt == nkt - 1),
                            skip_group_check=True)

            def attention_hp(qc, hp):
                nkt = 4 * qc + 4
                ots = [otps.tile([65, 512], f32, tag="ot",
                                 name=f"ot{qc}_{hp}_{i}") for i in range(2)]
                pend = deque()
                for kp in range(nkt // 2):
                    cur = []
                    for i in (0, 1):
                        h = 2 * hp + i
                        tq = h // 2
                        po = (h % 2) * 64
                        qt_ap = qkt_sb[tq]
                        kt_ap = qkt_sb[2 + tq]
                        sp = bigps.tile([128, 1024], f32, tag="sp")
                        offs = []
                        for half in (0, 1):
                            kt = 2 * kp + half
                            di = kt - 4 * qc
                            off = 128 * di if di > 0 else 0
                            offs.append(off)
                            nc.tensor.matmul(
                                sp[:, half * 512 + off:(half + 1) * 512],
                                kt_ap[po:po + 64, kt * 128:(kt + 1) * 128],
                                qt_ap[po:po + 64,
                                      qc * 512 + off:(qc + 1) * 512],
                                start=True, stop=True)
                        slab = slabs.tile([128, 1024], bf16, tag="slab")
                        if offs[1] == 0:
                            nc.scalar.activation(slab[:], sp[:], FT.Exp)
                        else:
                            for half in (0, 1):
                                off = offs[half]
                                nc.scalar.activation(
                                    slab[:, half * 512 + off:(half + 1) * 512],
                                    sp[:, half * 512 + off:(half + 1) * 512],
                                    FT.Exp)
                        for half in (0, 1):
                            kt = 2 * kp + half
                            di = kt - 4 * qc
                            if di >= 0:
                                base = half * 512 + 128 * di
                                nc.vector.tensor_mul(
                                    slab[:, base:base + 128],
                                    slab[:, base:base + 128],
                                    mtri_sb[:])
                        cur.append((slab, kp))
                    pend.append(cur)
                    while len(pend) > 1:
                        emit_attnv(qc, hp, ots, pend.popleft())
                    pop_filler(900 if qc < 2 else 1300, qc >= 2)
                while pend:
                    emit_attnv(qc, hp, ots, pend.popleft())
                return ots

            def normalize_hp(qc, hp, ots):
                # normalize: rows 0..63 * (1/Z), Z = row 64
                for i in (0, 1):
                    h = 2 * hp + i
                    po = (h % 2) * 64
                    fine = (qc == 3 and hp == 1)
                    nh = 2 if fine else 1
                    w = 512 // nh
                    for half in range(nh):
                        zrow = small.tile([1, 512], f32, tag="zrow")
                        if fine:
                            nc.scalar.copy(
                                zrow[:, 0:w],
                                ots[i][64:65, half * w:half * w + w])
                        else:
                            nc.vector.tensor_copy(
                                zrow[:, 0:w],
                                ots[i][64:65, half * w:half * w + w])
                        rz = small.tile([1, 512], f32, tag="rz")
                        nc.vector.reciprocal_approx_fast(
                            rz[:, 0:w], zrow[:, 0:w])
                        sbb = small.tile([64, 512], f32, tag="sbb")
                        nc.gpsimd.partition_broadcast(
                            sbb[:, 0:w], rz[0:1, 0:w])
                        nc.vector.tensor_mul(
                            attnT_sb[h // 2][po:po + 64,
                                             qc * 512 + half * w:
                                             qc * 512 + half * w + w],
                            ots[i][0:64, half * w:half * w + w],
                            sbb[:, 0:w])

            # ---- pass A: projections needed by qc0 ----
            for u in [("v", 0), ("v", 1), ("qk", 0, 0), ("qk", 2, 0),
                      ("v", 2), ("v", 3), ("qk", 1, 0), ("qk", 3, 0)]:
                emit_unit(u)

            # filler queue: later projections in the order attention needs them
            for qc in range(1, 4):
                work.extend([("qk", 0, qc), ("qk", 2, qc),
                             ("v", 4 * qc), ("v", 4 * qc + 1),
                             ("qk", 1, qc), ("qk", 3, qc),
                             ("v", 4 * qc + 2), ("v", 4 * qc + 3)])

            for qc in range(NQC):
                # force anything this q-chunk depends on that wasn't drained
                for t in range(4):
                    emit_qk(t, qc)
                for st in range(4 * qc + 4):
                    emit_v(st)
                for hp in (0, 1):
                    ots = attention_hp(qc, hp)
                    if qc == 3 and hp == 1:
                        while cpwork:
                            emit_unit(cpwork.popleft())
                        for half in (0, 1):
                            for i in (0, 1):
                                h = 2 * hp + i
                                po = (h % 2) * 64
                                c0 = qc * 512 + half * 256
                                zrow = small.tile([1, 512], f32, tag="zrow")
                                nc.scalar.copy(
                                    zrow[:, 0:256],
                                    ots[i][64:65, half * 256:half * 256 + 256])
                                rz = small.tile([1, 512], f32, tag="rz")
                                nc.vector.reciprocal_approx_fast(
                                    rz[:, 0:256], zrow[:, 0:256])
                                sbb = small.tile([64, 512], f32, tag="sbb")
                                nc.gpsimd.partition_broadcast(
                                    sbb[:, 0:256], rz[0:1, 0:256])
                                nc.vector.tensor_mul(
                                    attnT_sb[h // 2][po:po + 64, c0:c0 + 256],
                                    ots[i][0:64, half * 256:half * 256 + 256],
                                    sbb[:, 0:256])
                            for sti in (2 * half, 2 * half + 1):
                                for nchk in (0, 1):
                                    emit_cp(12 + sti, nchk)
                    else:
                        normalize_hp(qc, hp, ots)
                # c_proj for this q-chunk: held back as late-stretch filler
                for sti in range(4):
                    for nchk in range(2):
                        cpwork.append(("cp", 4 * qc + sti, nchk))

            while work:
                emit_unit(work.popleft())
            while cpwork:
                emit_unit(cpwork.popleft())

    nc.finalize()
    return nc


def _prep_inputs(hidden_states, w_attn, b_attn, w_proj, b_proj):
    hs = np.asarray(hidden_states, np.float32)
    wa = np.asarray(w_attn, np.float32)
    ba = np.asarray(b_attn, np.float32)
    wpj = np.asarray(w_proj, np.float32)

    import ml_dtypes
    bfl = ml_dtypes.bfloat16
    xTs = [np.ascontiguousarray(hs[b].T.astype(bfl)) for b in range(B)]
    mtri = (np.arange(128)[:, None] <= np.arange(128)[None, :]).astype(bfl)

    in_maps = []
    for c in range(8):
        b, g = c // 4, c % 4
        q0 = DP * g
        k0 = E + DP * g
        v0 = 2 * E + DP * g
        wqk = np.concatenate(
            [wa[:, q0:q0 + DP] * 0.125, wa[:, k0:k0 + DP]], axis=1).astype(bfl)
        bqk = np.zeros((128, 4), np.float32)
        bqk[:, 0] = ba[q0:q0 + 128] * 0.125
        bqk[:, 1] = ba[q0 + 128:q0 + 256] * 0.125
        bqk[:, 2] = ba[k0:k0 + 128]
        bqk[:, 3] = ba[k0 + 128:k0 + 256]
        wv = np.zeros((E, 260), bfl)
        vb = np.zeros((128, 260), np.float32)
        for h in range(HPC):
            wv[:, 65 * h:65 * h + 64] = wa[:, v0 + 64 * h:v0 + 64 * h + 64].astype(bfl)
            vb[:, 65 * h:65 * h + 64] = ba[v0 + 64 * h:v0 + 64 * h + 64]
            vb[:, 65 * h + 64] = 1.0
        wp = np.ascontiguousarray(wpj[DP * g:DP * (g + 1), :].astype(bfl))
        in_maps.append({
            "xT": xTs[b],
            "wqk": np.ascontiguousarray(wqk),
            "bqk": bqk,
            "wv": wv,
            "vb": vb,
            "wp": wp,
            "mtri": mtri,
        })
    return in_maps


def run(trace=False, **inputs):
    if "nc" not in _CACHED:
        _CACHED["nc"] = build_nc()
    nc = _CACHED["nc"]
    in_maps = _prep_inputs(**inputs)
    res = run_bass_kernel_spmd(nc, in_maps, list(range(8)), trace=trace)
    b_proj = np.asarray(inputs["b_proj"], np.float32)
    out = np.empty((B, S, E), np.float32)
    for b in range(B):
        acc = res.results[4 * b]["outp"].astype(np.float32)
        for g in range(1, 4):
            acc = acc + res.results[4 * b + g]["outp"]
        out[b] = acc + b_proj
    return out, res


def kernel(**inputs):
    out, _ = run(trace=False, **inputs)
    return out
